# revision 2
# baseline (speedup 1.0000x reference)
"""Int8-quantized 3x3 conv (32->32 ch) on 8 trn2 NeuronCores — v2.

Sharding: batch-parallel, 1 image per core (B=8).

Host side: x_q = round(x/0.05f) (bit-exact vs the reference quantizer),
shipped as bf16 (ints |v|<=128, exact).  Weight tensor is repacked on
host into the matmul lhsT layout; bias replicated per partition.

Device side, per output row-pair (r, r+1):
  window = input rows r-1..r+2 as SBUF partitions (wr, ic) = 4x32 = 128
  3 matmuls (one per dx tap column) with K=128, M=64=(rr,oc), N<=512
  accumulate into one PSUM half; two pairs share a [128,512] PSUM tile.
Windows for 16 pairs live in one wide [128, 16*512] bf16 tile filled by
4 strided DMAs (row step 2 — each input row is loaded twice, trading
+16MB HBM for zero reshuffle work).

Epilogue per 4-row chunk (partition p = 32*row_in_chunk + oc):
  e1 = (psum + bias) * s      (DVE; exact int + one RNE mult)
  e2 = rint(e1) via +/-MAGIC  (DVE; RNE adds)
  e3 = clip(e2, 0, 127)       (GpSimd; exact)
  out = 0.1 * e3              (ACT; final op, 1-ulp-safe)
8 chunks stage into a [128, 8*512] f32 tile -> one DMA to DRAM.
"""

import numpy as np
from contextlib import ExitStack

import concourse.bass as bass
import concourse.tile as tile
from concourse import bacc, mybir
from concourse.bass_utils import run_bass_kernel_spmd

F32 = mybir.dt.float32
BF16 = mybir.dt.bfloat16
ALU = mybir.AluOpType
AFT = mybir.ActivationFunctionType

C = 32          # channels (in and out)
H = W = 512
P = 128         # SBUF partitions
NPAIR = H // 2  # 256 output row-pairs
J = 16          # row-pair windows per wide tile
G = 16          # 4-row chunks per output staging tile
MAGIC = 12582912.0                              # 1.5 * 2^23: fp32 rint trick
S_REQ = float(np.float32(0.05 * 0.02 / 0.1))    # 0.009999999776482582
S_OUT = float(np.float32(0.1))                  # 0.10000000149011612

_CACHE = {}


def _build_program():
    nc = bacc.Bacc(None, target_bir_lowering=False, debug=False)
    x_d = nc.declare_dram_parameter("x", [C, H, W], BF16, isOutput=False)
    wl_d = nc.declare_dram_parameter("wl", [P, 192], BF16, isOutput=False)
    b_d = nc.declare_dram_parameter("bb", [P, 1], F32, isOutput=False)
    y_d = nc.declare_dram_parameter("y", [C, H, W], F32, isOutput=True)

    # row = 2*hp + par  (parity-split view for step-2 row gathers)
    x_par = x_d.rearrange("c (hp two) w -> two c hp w", two=2)
    # row = 4*hq + rg   (parity-4 view for strided output stores)
    y_q4 = y_d.rearrange("o (hq four) w -> four o hq w", four=4)

    with tile.TileContext(nc) as tc, ExitStack() as ctx:
        const = ctx.enter_context(tc.tile_pool(name="const", bufs=1))
        spec_p = ctx.enter_context(tc.tile_pool(name="spec", bufs=1))
        wide_p = ctx.enter_context(tc.tile_pool(name="wide", bufs=5))
        e_p = ctx.enter_context(tc.tile_pool(name="epi", bufs=4))
        out_p = ctx.enter_context(tc.tile_pool(name="out", bufs=2))
        psum_p = ctx.enter_context(
            tc.tile_pool(name="psum", bufs=8, space=bass.MemorySpace.PSUM))

        # ---- constants ------------------------------------------------
        wl = const.tile([P, 192], BF16)
        nc.sync.dma_start(wl[:], wl_d[:])
        bb = const.tile([P, 1], F32)
        nc.sync.dma_start(bb[:], b_d[:])

        # ---- edge windows (pairs 0 and 255) ---------------------------
        s0 = spec_p.tile([P, W], BF16)           # rows -1,0,1,2 (row -1 = 0)
        nc.vector.memset(s0[0:C, :], 0.0)
        for r in range(3):
            nc.sync.dma_start(s0[C * (r + 1):C * (r + 2), :], x_d[:, r, :])
        s1 = spec_p.tile([P, W], BF16)           # rows 509,510,511,512(=0)
        for r in range(3):
            nc.sync.dma_start(s1[C * r:C * (r + 1), :], x_d[:, 509 + r, :])
        nc.vector.memset(s1[3 * C:P, :], 0.0)

        wide = {}

        def load_wide(t):
            """Wide tile t: windows for pairs 16t+1 .. 16t+16 (j = p-16t-1).
            Window j block wr holds input row 32t+1+wr+2j."""
            jn = min(J, 254 - (16 * t + 1) + 1)
            wt = wide_p.tile([P, J * W], BF16, tag="wide")
            for wr in range(4):
                a = 32 * t + 1 + wr
                nc.sync.dma_start(
                    wt[C * wr:C * (wr + 1), 0:jn * W].rearrange(
                        "c (j w) -> c j w", w=W),
                    x_par[a % 2][:, a // 2:a // 2 + jn, :])
            wide[t] = wt

        # ---- main loop: one 4-row chunk per iteration -----------------
        NT = (254 + J - 1) // J          # number of wide tiles
        for t0 in range(4):
            load_wide(t0)                # deep preload: cover cold-PE phase
        for q in range(H // 4):
            ps = psum_p.tile([P, W], F32)
            for half in range(2):
                p = 2 * q + half
                if p == 0:
                    src, base = s0, 0
                elif p == NPAIR - 1:
                    src, base = s1, 0
                else:
                    t, j = divmod(p - 1, J)
                    if j == 0 and t + 2 < NT and t + 2 not in wide:
                        load_wide(t + 2)   # prefetch two tiles ahead
                    if t not in wide:
                        load_wide(t)
                    src, base = wide[t], W * j
                pb = 64 * half
                # dx taps: center (full width, start), left, right (stop)
                nc.tensor.matmul(
                    ps[pb:pb + 64, 0:W],
                    wl[:, 64:128],
                    src[:, base:base + W],
                    start=True, stop=False, tile_position=(0, pb))
                nc.tensor.matmul(
                    ps[pb:pb + 64, 1:W],
                    wl[:, 0:64],
                    src[:, base:base + W - 1],
                    start=False, stop=False, tile_position=(0, pb))
                nc.tensor.matmul(
                    ps[pb:pb + 64, 0:W - 1],
                    wl[:, 128:192],
                    src[:, base + 1:base + W],
                    start=False, stop=True, tile_position=(0, pb))

            # epilogue: y = 0.1 * clip(rint(s*(psum+bias)), 0, 127)
            e1 = e_p.tile([P, W], F32, tag="e1")
            nc.vector.tensor_scalar(e1[:], ps[:], bb[:, 0:1], S_REQ,
                                    ALU.add, ALU.mult)
            e2 = e_p.tile([P, W], F32, tag="e2")
            nc.vector.tensor_scalar(e2[:], e1[:], MAGIC, MAGIC,
                                    ALU.add, ALU.subtract)
            e3 = e_p.tile([P, W], F32, tag="e3")
            # alternate clamp between GpSimd and DVE to balance engines
            eng = nc.gpsimd if q % 2 == 0 else nc.vector
            eng.tensor_scalar(e3[:], e2[:], 0.0, 127.0,
                              ALU.max, ALU.min)
            g = q % G
            if g == 0:
                ow = out_p.tile([P, G * W], F32, tag="ow")
            nc.scalar.activation(ow[:, g * W:(g + 1) * W], e3[:],
                                 AFT.Identity, scale=S_OUT)
            if g == G - 1:
                # SWDGE (Pool) path: separate semaphore pool, so stores
                # never share a DMAHW lane with — and thus stall — the
                # SP input prefetch stream.
                q0 = q - G + 1
                for rg in range(4):
                    nc.gpsimd.dma_start(
                        y_q4[rg][:, q0:q0 + G, :],
                        ow[C * rg:C * (rg + 1), :].rearrange(
                            "c (g w) -> c g w", w=W))

    nc.compile()
    return nc


def kernel(x_float, weight, bias):
    if "nc" not in _CACHE:
        _CACHE["nc"] = _build_program()
    nc = _CACHE["nc"]
    np_bf16 = mybir.dt.np(BF16)

    x = np.asarray(x_float, dtype=np.float32)
    w = np.asarray(weight, dtype=np.int64)
    b = np.asarray(bias, dtype=np.int64)

    # exact reference quantizer: round(x/0.05f) + zp, clip, recenter
    xq = np.clip(np.round(x / np.float32(0.05)) + 128, 0, 255) - 128
    xq = np.ascontiguousarray(xq.astype(np_bf16))

    # lhsT: [k=(wr,ic), m=(dx,rr,oc)] = w[oc,ic,wr-rr,dx] - 128
    wlmat = np.zeros((P, 192), np.float32)
    for dx in range(3):
        for rr in range(2):
            for dy in range(3):
                wlmat[32 * (rr + dy):32 * (rr + dy + 1),
                      64 * dx + 32 * rr:64 * dx + 32 * rr + 32] = \
                    (w[:, :, dy, dx].T - 128).astype(np.float32)
    wlmat = wlmat.astype(np_bf16)

    bb = np.tile(b.astype(np.float32), 4).reshape(P, 1)

    n_cores = x.shape[0]
    in_maps = [{"x": xq[i], "wl": wlmat, "bb": bb} for i in range(n_cores)]
    res = run_bass_kernel_spmd(nc, in_maps, core_ids=list(range(n_cores)))
    out = np.stack([res.results[i]["y"] for i in range(n_cores)], axis=0)
    return out.astype(np.float32)


# revision 3
# speedup vs baseline: 3.5216x; 3.5216x over previous
"""Int8-quantized 3x3 conv (32->32 ch) on 8 trn2 NeuronCores — v2.

Sharding: batch-parallel, 1 image per core (B=8).

Host side: x_q = round(x/0.05f) (bit-exact vs the reference quantizer),
shipped as bf16 (ints |v|<=128, exact).  Weight tensor is repacked on
host into the matmul lhsT layout; bias replicated per partition.

Device side, per output row-pair (r, r+1):
  window = input rows r-1..r+2 as SBUF partitions (wr, ic) = 4x32 = 128
  3 matmuls (one per dx tap column) with K=128, M=64=(rr,oc), N<=512
  accumulate into one PSUM half; two pairs share a [128,512] PSUM tile.
Windows for 16 pairs live in one wide [128, 16*512] bf16 tile filled by
4 strided DMAs (row step 2 — each input row is loaded twice, trading
+16MB HBM for zero reshuffle work).

Epilogue per 4-row chunk (partition p = 32*row_in_chunk + oc):
  e1 = (psum + bias) * s      (DVE; exact int + one RNE mult)
  e2 = rint(e1) via +/-MAGIC  (DVE; RNE adds)
  e3 = clip(e2, 0, 127)       (GpSimd; exact)
  out = 0.1 * e3              (ACT; final op, 1-ulp-safe)
8 chunks stage into a [128, 8*512] f32 tile -> one DMA to DRAM.
"""

import numpy as np
from contextlib import ExitStack

import concourse.bass as bass
import concourse.tile as tile
from concourse import bacc, mybir
from concourse.bass_utils import run_bass_kernel_spmd

F32 = mybir.dt.float32
BF16 = mybir.dt.bfloat16
ALU = mybir.AluOpType
AFT = mybir.ActivationFunctionType

C = 32          # channels (in and out)
H = W = 512
P = 128         # SBUF partitions
NPAIR = H // 2  # 256 output row-pairs
J = 16          # row-pair windows per wide tile
G = 16          # 4-row chunks per output staging tile
MAGIC = 12582912.0                              # 1.5 * 2^23: fp32 rint trick
S_REQ = float(np.float32(0.05 * 0.02 / 0.1))    # 0.009999999776482582
S_OUT = float(np.float32(0.1))                  # 0.10000000149011612

_CACHE = {}


def _build_program():
    nc = bacc.Bacc(None, target_bir_lowering=False, debug=False)
    x_d = nc.declare_dram_parameter("x", [C, H, W], BF16, isOutput=False)
    wl_d = nc.declare_dram_parameter("wl", [P, 192], BF16, isOutput=False)
    b_d = nc.declare_dram_parameter("bb", [P, 1], F32, isOutput=False)
    y_d = nc.declare_dram_parameter("y", [C, H, W], F32, isOutput=True)

    # row = 2*hp + par  (parity-split view for step-2 row gathers)
    x_par = x_d.rearrange("c (hp two) w -> two c hp w", two=2)
    # row = 4*hq + rg   (parity-4 view for strided output stores)
    y_q4 = y_d.rearrange("o (hq four) w -> four o hq w", four=4)

    with tile.TileContext(nc) as tc, ExitStack() as ctx:
        const = ctx.enter_context(tc.tile_pool(name="const", bufs=1))
        spec_p = ctx.enter_context(tc.tile_pool(name="spec", bufs=1))
        wide_p = ctx.enter_context(tc.tile_pool(name="wide", bufs=6))
        e_p = ctx.enter_context(tc.tile_pool(name="epi", bufs=4))
        out_p = ctx.enter_context(tc.tile_pool(name="out", bufs=2))
        psum_p = ctx.enter_context(
            tc.tile_pool(name="psum", bufs=7, space=bass.MemorySpace.PSUM))
        warm_p = ctx.enter_context(
            tc.tile_pool(name="warm", bufs=1, space=bass.MemorySpace.PSUM))

        # PE warm-up: dead matmuls on a zeroed scratch tile keep the PE
        # busy through its p-state ramp while the first windows load.
        # No DMA dependencies, so the ramp starts immediately.
        zscr = const.tile([P, 128], BF16)
        nc.vector.memset(zscr[:], 0.0)
        wps = warm_p.tile([P, 128], F32, tag="warm")
        for _ in range(40):
            nc.tensor.matmul(wps[0:64, :], zscr[:, 0:64], zscr[:, :],
                             start=True, stop=True)

        wide = {}

        def load_wide(t):
            """Wide tile t: windows for pairs 16t+1 .. 16t+16 (j = p-16t-1).
            Window j block wr holds input row 32t+1+wr+2j."""
            jn = min(J, 254 - (16 * t + 1) + 1)
            wt = wide_p.tile([P, J * W], BF16, tag="wide")
            for wr in range(4):
                a = 32 * t + 1 + wr
                nc.sync.dma_start(
                    wt[C * wr:C * (wr + 1), 0:jn * W].rearrange(
                        "c (j w) -> c j w", w=W),
                    x_par[a % 2][:, a // 2:a // 2 + jn, :])
            wide[t] = wt

        # ---- main loop: one 4-row chunk per iteration -----------------
        # output groups: big in steady state, tapering at the end so the
        # final stores aren't serialized behind one long epilogue chain
        group_of = {}
        q0 = 0
        for gq in [16] * 7 + [8, 4, 4]:
            for g in range(gq):
                group_of[q0 + g] = (q0, gq, g)
            q0 += gq

        # ---- constants + edge windows on the ACT DMA queue, so the SP
        # queue starts the wide input stream at t=0 in parallel ---------
        wl = const.tile([P, 192], BF16)
        nc.scalar.dma_start(wl[:], wl_d[:])
        bb = const.tile([P, 1], F32)
        nc.scalar.dma_start(bb[:], b_d[:])
        s0 = spec_p.tile([P, W], BF16)           # rows -1,0,1,2 (row -1 = 0)
        nc.vector.memset(s0[0:C, :], 0.0)
        for r in range(3):
            nc.scalar.dma_start(s0[C * (r + 1):C * (r + 2), :], x_d[:, r, :])
        s1 = spec_p.tile([P, W], BF16)           # rows 509,510,511,512(=0)
        for r in range(3):
            nc.scalar.dma_start(s1[C * r:C * (r + 1), :], x_d[:, 509 + r, :])
        nc.vector.memset(s1[3 * C:P, :], 0.0)

        NT = (254 + J - 1) // J          # number of wide tiles
        for t0 in range(5):
            load_wide(t0)                # deep preload: cover cold-PE phase
        for q in range(H // 4):
            ps = psum_p.tile([P, W], F32)
            for half in range(2):
                p = 2 * q + half
                if p == 0:
                    src, base = s0, 0
                elif p == NPAIR - 1:
                    src, base = s1, 0
                else:
                    t, j = divmod(p - 1, J)
                    if j == 0 and t + 2 < NT and t + 2 not in wide:
                        load_wide(t + 2)   # prefetch two tiles ahead
                    if j == J // 2 and t + 3 < NT and t + 3 not in wide:
                        load_wide(t + 3)   # half-tile cadence
                    if t not in wide:
                        load_wide(t)
                    src, base = wide[t], W * j
                pb = 64 * half
                # dx taps: center (full width, start), left, right (stop)
                nc.tensor.matmul(
                    ps[pb:pb + 64, 0:W],
                    wl[:, 64:128],
                    src[:, base:base + W],
                    start=True, stop=False, tile_position=(0, pb))
                nc.tensor.matmul(
                    ps[pb:pb + 64, 1:W],
                    wl[:, 0:64],
                    src[:, base:base + W - 1],
                    start=False, stop=False, tile_position=(0, pb))
                nc.tensor.matmul(
                    ps[pb:pb + 64, 0:W - 1],
                    wl[:, 128:192],
                    src[:, base + 1:base + W],
                    start=False, stop=True, tile_position=(0, pb))

            # epilogue: y = 0.1 * clip(rint(s*(psum+bias)), 0, 127)
            e1 = e_p.tile([P, W], F32, tag="e1")
            nc.vector.tensor_scalar(e1[:], ps[:], bb[:, 0:1], S_REQ,
                                    ALU.add, ALU.mult)
            e2 = e_p.tile([P, W], F32, tag="e2")
            nc.vector.tensor_scalar(e2[:], e1[:], MAGIC, MAGIC,
                                    ALU.add, ALU.subtract)
            e3 = e_p.tile([P, W], F32, tag="e3")
            # alternate clamp between GpSimd and DVE to balance engines
            eng = nc.gpsimd if q % 2 == 0 else nc.vector
            eng.tensor_scalar(e3[:], e2[:], 0.0, 127.0,
                              ALU.max, ALU.min)
            q0, gq, g = group_of[q]
            if g == 0:
                ow = out_p.tile([P, gq * W], F32, tag="ow")
            nc.scalar.activation(ow[:, g * W:(g + 1) * W], e3[:],
                                 AFT.Identity, scale=S_OUT)
            if g == gq - 1:
                # SWDGE (Pool) path: separate semaphore pool, so stores
                # never share a DMAHW lane with — and thus stall — the
                # SP input prefetch stream.
                for rg in range(4):
                    nc.gpsimd.dma_start(
                        y_q4[rg][:, q0:q0 + gq, :],
                        ow[C * rg:C * (rg + 1), 0:gq * W].rearrange(
                            "c (g w) -> c g w", w=W))

    nc.compile()
    return nc


def kernel(x_float, weight, bias):
    if "nc" not in _CACHE:
        _CACHE["nc"] = _build_program()
    nc = _CACHE["nc"]
    np_bf16 = mybir.dt.np(BF16)

    x = np.asarray(x_float, dtype=np.float32)
    w = np.asarray(weight, dtype=np.int64)
    b = np.asarray(bias, dtype=np.int64)

    # exact reference quantizer: round(x/0.05f) + zp, clip, recenter
    xq = np.clip(np.round(x / np.float32(0.05)) + 128, 0, 255) - 128
    xq = np.ascontiguousarray(xq.astype(np_bf16))

    # lhsT: [k=(wr,ic), m=(dx,rr,oc)] = w[oc,ic,wr-rr,dx] - 128
    wlmat = np.zeros((P, 192), np.float32)
    for dx in range(3):
        for rr in range(2):
            for dy in range(3):
                wlmat[32 * (rr + dy):32 * (rr + dy + 1),
                      64 * dx + 32 * rr:64 * dx + 32 * rr + 32] = \
                    (w[:, :, dy, dx].T - 128).astype(np.float32)
    wlmat = wlmat.astype(np_bf16)

    bb = np.tile(b.astype(np.float32), 4).reshape(P, 1)

    n_cores = x.shape[0]
    in_maps = [{"x": xq[i], "wl": wlmat, "bb": bb} for i in range(n_cores)]
    res = run_bass_kernel_spmd(nc, in_maps, core_ids=list(range(n_cores)))
    out = np.stack([res.results[i]["y"] for i in range(n_cores)], axis=0)
    return out.astype(np.float32)
